# revision 17
# baseline (speedup 1.0000x reference)
"""Trainium2 Bass kernel for GridSmoother.

Solves (I + Dx^T Wx Dx + Dy^T Wy Dy) x = ae per (batch, channel) with a
Jacobi-preconditioned Chebyshev semi-iteration (Golub-Varga omega form,
K matvec iterations), data-parallel over batch on 8 cores.

Per-core layout: SBUF tiles [H=120 partitions, (b=4, c=16, w=160) = 10240
free], bf16 storage.  The y-direction stencil runs on TensorE with fixed
[120x120] stationaries (Sy, I, -I, -Sy^T) + per-iteration scaled identities;
the x-direction stencil is shifted-AP DVE ops.  All per-iteration scalars
(omega_k, gamma) are folded into input tensors (stats matrix, prescaled
dinv slots), so the device program is data-independent and compiled once.

Recurrence (x0 = 0):
    x1 = gamma * dinv . b ;  d1 = x1
    for k = 1..K:
        r    = b - A x_k                     (psum passes + shifted DVE ops)
        d_k1 = (om_k - 1) d_k + om_k gamma dinv . r
        x_k1 = x_k + d_k1
"""
import numpy as np

B, C, H, W = 32, 16, 120, 160
NCORES = 8
BL = B // NCORES            # 4 batches/core
FREE = BL * C * W           # 10240
ITEM = C * W                # 2560 cols per batch item
GCH = 4                     # channels per psum group
GCOLS = GCH * W             # 640
NG = FREE // GCOLS          # 16 groups
K_ITERS = 6

_cache = {}


def _cheb_omegas(lmin, lmax, K):
    """Golub-Varga omega schedule + gamma."""
    gam = 2.0 / (lmax + lmin)
    sig = (lmax - lmin) / (lmax + lmin)
    oms = []
    om = 1.0
    for k in range(K):
        om = 2.0 / (2.0 - sig * sig) if k == 0 else \
            1.0 / (1.0 - 0.25 * sig * sig * om)
        oms.append(om)
    return gam, oms


def _host_stats(oms, K):
    """Stationary matrices stacked along free dim: [H, (4+K)*H] bf16.
    matmul computes lhsT.T @ rhs, so each slot stores M.T for operator M.
    slot 0: Sy      (Sy x)[h] = x[h+1]-x[h], row H-1 = 0
    slot 1: I
    slot 2: -I
    slot 3: -Sy^T
    slot 4+k: (om_k - 1) I
    """
    import ml_dtypes
    Sy = np.zeros((H, H), np.float32)
    for h in range(H - 1):
        Sy[h, h] = -1.0
        Sy[h, h + 1] = 1.0
    I = np.eye(H, dtype=np.float32)
    mats = np.zeros((H, (4 + K) * H), np.float32)
    mats[:, 0:H] = Sy.T
    mats[:, H:2 * H] = I
    mats[:, 2 * H:3 * H] = -I
    mats[:, 3 * H:4 * H] = (-Sy.T).T          # stationary for -Sy^T is -Sy
    for k in range(K):
        mats[:, (4 + k) * H:(5 + k) * H] = np.float32(oms[k] - 1.0) * I
    return mats.astype(ml_dtypes.bfloat16)


def _build(K):
    import sys
    if '/opt/trn_rl_repo' not in sys.path:
        sys.path.insert(0, '/opt/trn_rl_repo')
    from contextlib import ExitStack
    import concourse.bass as bass
    import concourse.tile as tile
    from concourse import bacc, mybir

    f32 = mybir.dt.float32
    bf16 = mybir.dt.bfloat16
    ALU = mybir.AluOpType

    nc = bacc.Bacc("TRN2", target_bir_lowering=False, debug=False,
                   num_devices=NCORES)
    # all inputs host-prepped, h-major
    b_ap = nc.dram_tensor("bh", [H, BL, C, W], bf16, kind="ExternalInput").ap()
    wx_ap = nc.dram_tensor("wxh", [H, BL, W], bf16, kind="ExternalInput").ap()
    wy_ap = nc.dram_tensor("wyh", [H, BL, W], bf16, kind="ExternalInput").ap()
    dk_ap = nc.dram_tensor("dinvk", [H, K + 1, BL, W], bf16,
                           kind="ExternalInput").ap()
    st_ap = nc.dram_tensor("stats", [H, (4 + K) * H], bf16,
                           kind="ExternalInput").ap()
    om_ap = nc.dram_tensor("omv", [H, K], f32, kind="ExternalInput").ap()
    out_ap = nc.dram_tensor("out", [H, BL, C, W], bf16,
                            kind="ExternalOutput").ap()

    with tile.TileContext(nc) as tc, ExitStack() as ctx:
        per = ctx.enter_context(tc.tile_pool(name="per", bufs=1))
        t2p = ctx.enter_context(tc.tile_pool(name="t2p", bufs=3))
        pa = ctx.enter_context(tc.tile_pool(name="pa", bufs=2, space="PSUM"))
        pb = ctx.enter_context(tc.tile_pool(name="pb", bufs=2, space="PSUM"))

        bt = per.tile([H, FREE], bf16, tag="bt")
        xa = per.tile([H, FREE], bf16, tag="xa")
        dt = per.tile([H, FREE], bf16, tag="dt")
        tp = per.tile([H, FREE + W], bf16, tag="tp")   # t, lead pad col
        ut = per.tile([H, FREE], bf16, tag="ut")       # psumB evac
        mt = per.tile([H, FREE], bf16, tag="mt")       # m / z scratch
        wxt = per.tile([H, BL * W], bf16, tag="wxt")
        wyt = per.tile([H, BL * W], bf16, tag="wyt")
        dkt = per.tile([H, (K + 1) * BL * W], bf16, tag="dkt")
        stt = per.tile([H, (4 + K) * H], bf16, tag="stt")
        omt = per.tile([H, K], f32, tag="omt")
        ot = per.tile([H, FREE], bf16, tag="ot")

        # ---- loads ----
        nc.sync.dma_start(stt[:], st_ap[:])
        nc.sync.dma_start(omt[:], om_ap[:])
        nc.sync.dma_start(wxt[:].rearrange('h (b w) -> h b w', b=BL), wx_ap[:])
        nc.sync.dma_start(wyt[:].rearrange('h (b w) -> h b w', b=BL), wy_ap[:])
        nc.sync.dma_start(
            dkt[:].rearrange('h (k b w) -> h k b w', k=K + 1, b=BL), dk_ap[:])
        b4 = bt[:].rearrange('h (b c w) -> h b c w', b=BL, c=C)
        for i in range(BL):
            eng = nc.scalar if i % 2 else nc.sync
            eng.dma_start(b4[:, i], b_ap[:, i])

        # zero only the pad slots: cols c*W for c in 0..BL*C (stride-W view)
        nc.vector.memset(
            tp[:].rearrange('h (c w) -> h c w', w=W)[:, :, 0:1], 0.0)

        stat_sy = stt[:, 0:H]
        stat_i = stt[:, H:2 * H]
        stat_ni = stt[:, 2 * H:3 * H]
        stat_nsyt = stt[:, 3 * H:4 * H]

        wyr = wyt[:].rearrange('h (b w) -> h b w', b=BL)
        wxr = wxt[:].rearrange('h (b w) -> h b w', b=BL)
        dkr = dkt[:].rearrange('h (k b w) -> h k b w', k=K + 1, b=BL)

        def item_view(t, i, pad=0):
            return t[:, pad + i * ITEM: pad + (i + 1) * ITEM]

        # ---- prologue: x1 = gamma * dinv . b ; d = x1 ----
        for i in range(BL):
            d0 = dkr[:, 0, i].unsqueeze(1).broadcast_to([H, C, W])
            bv = item_view(bt, i).rearrange('h (c w) -> h c w', c=C)
            nc.vector.tensor_tensor(
                item_view(xa, i).rearrange('h (c w) -> h c w', c=C),
                bv, d0, ALU.mult)
            nc.scalar.copy(item_view(dt, i), item_view(xa, i))

        for k in range(K):
            xs = xa
            last = (k == K - 1)

            # x-direction: t[w] = wx[w]*(x[w+1]-x[w]), w in [0,158] per chan
            for i in range(BL):
                xv = item_view(xs, i).rearrange('h (c w) -> h c w', c=C)
                tv = item_view(tp, i, pad=1).rearrange('h (c w) -> h c w', c=C)
                nc.vector.tensor_tensor(tv[:, :, 0:W - 1], xv[:, :, 1:W],
                                        xv[:, :, 0:W - 1], ALU.subtract)
                wxb = (wxr[:, i, 0:W - 1].unsqueeze(1)
                       .broadcast_to([H, C, W - 1]))
                nc.vector.tensor_tensor(tv[:, :, 0:W - 1], tv[:, :, 0:W - 1],
                                        wxb, ALU.mult)

            for g in range(NG):
                i, cg = divmod(g, C // GCH)
                off = g * GCOLS
                xg = xs[:, off:off + GCOLS]
                # psumA = Sy @ x
                ga = pa.tile([H, GCOLS], f32, tag="ga")
                nc.tensor.matmul(ga[:, 0:512], stat_sy, xg[:, 0:512],
                                 start=True, stop=True)
                nc.tensor.matmul(ga[:, 512:GCOLS], stat_sy, xg[:, 512:GCOLS],
                                 start=True, stop=True)
                # t2 = psumA * wy   (broadcast over GCH channels)
                # (GPSIMD cannot read PSUM on HW: evac via Act first)
                a0 = t2p.tile([H, GCOLS], bf16, tag="a0")
                nc.scalar.copy(a0[:], ga[:])
                t2 = t2p.tile([H, GCOLS], bf16, tag="t2")
                wyb = (wyr[:, i].unsqueeze(1).broadcast_to([H, GCH, W]))
                nc.gpsimd.tensor_tensor(
                    t2[:].rearrange('h (c w) -> h c w', c=GCH),
                    a0[:].rearrange('h (c w) -> h c w', c=GCH),
                    wyb, ALU.mult)
                # psumB = I@b + I@t_cur - I@x - I@t_prev - SyT@t2  (= b - Ax)
                gb = pb.tile([H, GCOLS], f32, tag="gb")
                for lo, hi in ((0, 512), (512, GCOLS)):
                    nc.tensor.matmul(gb[:, lo:hi], stat_i,
                                     bt[:, off + lo:off + hi],
                                     start=True, stop=False)
                    nc.tensor.matmul(gb[:, lo:hi], stat_i,
                                     tp[:, 1 + off + lo:1 + off + hi],
                                     start=False, stop=False)
                    nc.tensor.matmul(gb[:, lo:hi], stat_ni,
                                     xs[:, off + lo:off + hi],
                                     start=False, stop=False)
                    nc.tensor.matmul(gb[:, lo:hi], stat_ni,
                                     tp[:, off + lo:off + hi],
                                     start=False, stop=False)
                    nc.tensor.matmul(gb[:, lo:hi], stat_nsyt, t2[:, lo:hi],
                                     start=False, stop=True)
                # evac (r = b - Ax, bf16)
                nc.scalar.copy(ut[:, off:off + GCOLS], gb[:])

            # z = r * (om*gam*dinv) [Pool]; d = (om-1)*d + z [DVE stt]
            HC = C // 2
            for i in range(BL):
                for h2 in range(2):
                    sl = slice(i * ITEM + h2 * (ITEM // 2),
                               i * ITEM + (h2 + 1) * (ITEM // 2))
                    dkb = (dkr[:, k + 1, i].unsqueeze(1)
                           .broadcast_to([H, HC, W]))
                    nc.gpsimd.tensor_tensor(
                        mt[:, sl].rearrange('h (c w) -> h c w', c=HC),
                        ut[:, sl].rearrange('h (c w) -> h c w', c=HC),
                        dkb, ALU.mult)
                    nc.vector.scalar_tensor_tensor(
                        dt[:, sl], dt[:, sl], omt[:, k:k + 1],
                        mt[:, sl], ALU.mult, ALU.add)

            # x_{k+1} = x_k + d   (in-place DMA accumulate; last iter -> f32)
            for i in range(BL):
                if last:
                    nc.vector.tensor_tensor(item_view(ot, i), item_view(xs, i),
                                            item_view(dt, i), ALU.add)
                    o4 = ot[:].rearrange('h (b c w) -> h b c w', b=BL, c=C)
                    nc.sync.dma_start(out_ap[:, i], o4[:, i])
                else:
                    nc.vector.tensor_tensor(item_view(xa, i),
                                            item_view(xa, i),
                                            item_view(dt, i), ALU.add)

    nc.compile()
    return nc


def _get_program(K):
    if K not in _cache:
        _cache[K] = _build(K)
    return _cache[K]


def _host_prep(ae, wxwy, K):
    """Spectral bounds, schedules, per-core h-major bf16 inputs."""
    import ml_dtypes
    bf = ml_dtypes.bfloat16
    ae = np.ascontiguousarray(ae, np.float32)
    wxwy = np.ascontiguousarray(wxwy, np.float32)
    wx = wxwy[:, 0]
    wy = wxwy[:, 1]

    d = np.ones((B, H, W), np.float32)
    d[:, :, 1:] += wx[:, :, :-1]
    d[:, :, :-1] += wx[:, :, :-1]
    d[:, 1:, :] += wy[:, :-1, :]
    d[:, :-1, :] += wy[:, :-1, :]
    dinv = 1.0 / d
    dis = np.sqrt(dinv)

    def op_precond(v):  # D^-1/2 A D^-1/2, v: [B,H,W]
        u = dis * v
        dx = u[:, :, 1:] - u[:, :, :-1]
        dy = u[:, 1:, :] - u[:, :-1, :]
        wdx = wx[:, :, :-1] * dx
        wdy = wy[:, :-1, :] * dy
        out = u.copy()
        out[:, :, 1:] += wdx
        out[:, :, :-1] -= wdx
        out[:, 1:, :] += wdy
        out[:, :-1, :] -= wdy
        return dis * out

    rng = np.random.default_rng(3)
    v = rng.standard_normal((B, H, W)).astype(np.float32)
    for _ in range(30):
        av = op_precond(v)
        v = av / np.sqrt((av * av).sum(axis=(1, 2), keepdims=True))
    lmax = float(((v * op_precond(v)).sum(axis=(1, 2))).max())
    s = lmax + 0.05
    v = rng.standard_normal((B, H, W)).astype(np.float32)
    for _ in range(40):
        av = s * v - op_precond(v)
        v = av / np.sqrt((av * av).sum(axis=(1, 2), keepdims=True))
    lmin = s - float(((v * (s * v - op_precond(v))).sum(axis=(1, 2))).max())
    lmax *= 1.005
    lmin = max(1.0 / float(d.max()), lmin * 0.995)

    gam, oms = _cheb_omegas(lmin, lmax, K)
    stats = _host_stats(oms, K)

    # dinvk slots: 0 -> gamma*dinv, k+1 -> om_k*gamma*dinv   [B,K+1,H,W]
    dk = np.empty((B, K + 1, H, W), np.float32)
    dk[:, 0] = gam * dinv
    for k in range(K):
        dk[:, k + 1] = oms[k] * gam * dinv

    # per-partition scalars (om_k - 1) for the d-update stt
    omv = np.tile(np.float32([om - 1.0 for om in oms]), (H, 1))

    # h-major transposes
    bh = np.ascontiguousarray(ae.transpose(2, 0, 1, 3)).astype(bf)  # [H,B,C,W]
    wxh = np.ascontiguousarray(wx.transpose(1, 0, 2)).astype(bf)    # [H,B,W]
    wyh = np.ascontiguousarray(wy.transpose(1, 0, 2)).astype(bf)
    dkh = np.ascontiguousarray(dk.transpose(2, 1, 0, 3)).astype(bf)  # [H,K+1,B,W]

    in_maps = []
    for c in range(NCORES):
        sl = slice(c * BL, (c + 1) * BL)
        in_maps.append({
            "bh": np.ascontiguousarray(bh[:, sl]),
            "wxh": np.ascontiguousarray(wxh[:, sl]),
            "wyh": np.ascontiguousarray(wyh[:, sl]),
            "dinvk": np.ascontiguousarray(dkh[:, :, sl]),
            "stats": stats,
            "omv": omv,
        })
    return in_maps


def postprocess_core(out_core):
    """[H,BL,C,W] f32 -> [BL,C,H,W]"""
    return np.ascontiguousarray(out_core.transpose(1, 2, 0, 3))


def prepare(ae, wxwy):
    nc = _get_program(K_ITERS)
    in_maps = _host_prep(ae, wxwy, K_ITERS)
    return {"nc": nc, "in_maps": in_maps,
            "postprocess_core": lambda o, c: postprocess_core(o)}


def kernel(ae, wxwy):
    import sys
    if '/opt/trn_rl_repo' not in sys.path:
        sys.path.insert(0, '/opt/trn_rl_repo')
    from concourse.bass_utils import run_bass_kernel_spmd

    p = prepare(ae, wxwy)
    res = run_bass_kernel_spmd(p["nc"], p["in_maps"], list(range(NCORES)))
    out = np.concatenate(
        [postprocess_core(np.asarray(res.results[c]["out"]).reshape(H, BL, C, W))
         for c in range(NCORES)], axis=0)
    return out.astype(np.float32)


# revision 39
# speedup vs baseline: 1.2397x; 1.2397x over previous
"""Trainium2 Bass kernel for GridSmoother.

Solves (I + Dx^T Wx Dx + Dy^T Wy Dy) x = ae per (batch, channel) with a
Jacobi-preconditioned Chebyshev semi-iteration (Golub-Varga omega form,
K matvec iterations), data-parallel over batch on 8 cores.

Per-core layout: SBUF tiles [H=120 partitions, (b=4, c=16, w=160) = 10240
free], bf16 storage.  The y-direction stencil and most of the residual
assembly run on TensorE with fixed [120x120] stationaries (Sy, I, -I,
-Sy^T) accumulating into PSUM; the x-direction stencil is shifted-AP DVE
tensor_tensor ops; PSUM evacuation on the Activation engine; the
dinv/omega scalings on Pool.  All per-iteration scalars (omega_k, gamma)
are folded into input tensors (stats, omv, prescaled dinvk slots), so the
device program is data-independent and compiled once.

Recurrence (x0 = 0):
    x1 = gamma * dinv . b ;  d1 = x1
    for k = 1..K:
        r    = b - A x_k                     (psum passes + shifted DVE ops)
        d_k1 = (om_k - 1) d_k + om_k gamma dinv . r
        x_k1 = x_k + d_k1
"""
import numpy as np

B, C, H, W = 32, 16, 120, 160
NCORES = 8
BL = B // NCORES            # 4 batches/core
FREE = BL * C * W           # 10240
ITEM = C * W                # 2560 cols per batch item
GCH = 4                     # channels per psum group
GCOLS = GCH * W             # 640
NG = FREE // GCOLS          # 16 groups
K_ITERS = 5

_cache = {}


def _cheb_omegas(lmin, lmax, K):
    """Golub-Varga omega schedule + gamma."""
    gam = 2.0 / (lmax + lmin)
    sig = (lmax - lmin) / (lmax + lmin)
    oms = []
    om = 1.0
    for k in range(K):
        om = 2.0 / (2.0 - sig * sig) if k == 0 else \
            1.0 / (1.0 - 0.25 * sig * sig * om)
        oms.append(om)
    return gam, oms


def _host_stats(oms, K):
    """Stationary matrices stacked along free dim: [H, 4*H] bf16.
    matmul computes lhsT.T @ rhs, so each slot stores M.T for operator M.
    slot 0: Sy      (Sy x)[h] = x[h+1]-x[h], row H-1 = 0
    slot 1: I
    slot 2: -I
    slot 3: -Sy^T
    """
    import ml_dtypes
    Sy = np.zeros((H, H), np.float32)
    for h in range(H - 1):
        Sy[h, h] = -1.0
        Sy[h, h + 1] = 1.0
    I = np.eye(H, dtype=np.float32)
    mats = np.zeros((H, 4 * H), np.float32)
    mats[:, 0:H] = Sy.T
    mats[:, H:2 * H] = I
    mats[:, 2 * H:3 * H] = -I
    mats[:, 3 * H:4 * H] = (-Sy.T).T          # stationary for -Sy^T is -Sy
    return mats.astype(ml_dtypes.bfloat16)


def _build(K):
    import sys
    if '/opt/trn_rl_repo' not in sys.path:
        sys.path.insert(0, '/opt/trn_rl_repo')
    from contextlib import ExitStack
    import concourse.bass as bass
    import concourse.tile as tile
    from concourse import bacc, mybir

    f32 = mybir.dt.float32
    bf16 = mybir.dt.bfloat16
    ALU = mybir.AluOpType

    nc = bacc.Bacc("TRN2", target_bir_lowering=False, debug=False,
                   num_devices=NCORES)
    # all inputs host-prepped, h-major
    b_ap = nc.dram_tensor("bh", [H, BL, C, W], bf16, kind="ExternalInput").ap()
    wx_ap = nc.dram_tensor("wxh", [H, BL, W], bf16, kind="ExternalInput").ap()
    wy_ap = nc.dram_tensor("wyh", [H, BL, W], bf16, kind="ExternalInput").ap()
    dk_ap = nc.dram_tensor("dinvk", [H, K + 1, BL, W], bf16,
                           kind="ExternalInput").ap()
    st_ap = nc.dram_tensor("stats", [H, 4 * H], bf16,
                           kind="ExternalInput").ap()
    om_ap = nc.dram_tensor("omv", [H, K], f32, kind="ExternalInput").ap()
    out_ap = nc.dram_tensor("out", [H, BL, C, W], bf16,
                            kind="ExternalOutput").ap()

    with tile.TileContext(nc) as tc, ExitStack() as ctx:
        per = ctx.enter_context(tc.tile_pool(name="per", bufs=1))
        t2p = ctx.enter_context(tc.tile_pool(name="t2p", bufs=6))
        pa = ctx.enter_context(tc.tile_pool(name="pa", bufs=2, space="PSUM"))
        pb = ctx.enter_context(tc.tile_pool(name="pb", bufs=2, space="PSUM"))

        bt = per.tile([H, FREE], bf16, tag="bt")
        xa = per.tile([H, FREE], bf16, tag="xa")
        dt = per.tile([H, FREE], bf16, tag="dt")
        tp = per.tile([H, FREE + W], bf16, tag="tp")   # t, lead pad col
        ut = per.tile([H, FREE], bf16, tag="ut")       # psumB evac
        mt = per.tile([H, FREE], bf16, tag="mt")       # m / z scratch
        wxt = per.tile([H, BL * W], bf16, tag="wxt")
        wyt = per.tile([H, BL * W], bf16, tag="wyt")
        dkt = per.tile([H, (K + 1) * BL * W], bf16, tag="dkt")
        stt = per.tile([H, 4 * H], bf16, tag="stt")
        omt = per.tile([H, K], f32, tag="omt")
        ot = per.tile([H, FREE], bf16, tag="ot")

        # ---- loads ----
        nc.sync.dma_start(stt[:], st_ap[:])
        nc.sync.dma_start(omt[:], om_ap[:])
        nc.sync.dma_start(wxt[:].rearrange('h (b w) -> h b w', b=BL), wx_ap[:])
        nc.sync.dma_start(wyt[:].rearrange('h (b w) -> h b w', b=BL), wy_ap[:])
        nc.sync.dma_start(
            dkt[:].rearrange('h (k b w) -> h k b w', k=K + 1, b=BL), dk_ap[:])
        b4 = bt[:].rearrange('h (b c w) -> h b c w', b=BL, c=C)
        qengs = [nc.sync, nc.scalar, nc.sync, nc.scalar]
        for i in range(BL):
            qengs[i].dma_start(b4[:, i], b_ap[:, i])

        # zero only the pad slots: cols c*W for c in 0..BL*C (stride-W view)
        nc.vector.memset(
            tp[:].rearrange('h (c w) -> h c w', w=W)[:, :, 0:1], 0.0)

        stat_sy = stt[:, 0:H]
        stat_i = stt[:, H:2 * H]
        stat_ni = stt[:, 2 * H:3 * H]
        stat_nsyt = stt[:, 3 * H:4 * H]

        wyr = wyt[:].rearrange('h (b w) -> h b w', b=BL)
        wxr = wxt[:].rearrange('h (b w) -> h b w', b=BL)
        dkr = dkt[:].rearrange('h (k b w) -> h k b w', k=K + 1, b=BL)

        def item_view(t, i, pad=0):
            return t[:, pad + i * ITEM: pad + (i + 1) * ITEM]

        # ---- prologue: x1 = gamma * dinv . b ; d = x1 ----
        for i in range(BL):
            d0 = dkr[:, 0, i].unsqueeze(1).broadcast_to([H, C, W])
            bv = item_view(bt, i).rearrange('h (c w) -> h c w', c=C)
            nc.vector.tensor_tensor(
                item_view(xa, i).rearrange('h (c w) -> h c w', c=C),
                bv, d0, ALU.mult)
            nc.scalar.copy(item_view(dt, i), item_view(xa, i))

        for k in range(K):
            xs = xa
            last = (k == K - 1)

            # x-direction: t[w] = wx[w]*(x[w+1]-x[w]), w in [0,158] per chan
            for i in range(BL):
                for h2 in range(2):
                    base = i * ITEM + h2 * (ITEM // 2)
                    xv = xs[:, base:base + ITEM // 2].rearrange(
                        'h (c w) -> h c w', c=C // 2)
                    tv = tp[:, 1 + base:1 + base + ITEM // 2].rearrange(
                        'h (c w) -> h c w', c=C // 2)
                    nc.vector.tensor_tensor(tv[:, :, 0:W - 1], xv[:, :, 1:W],
                                            xv[:, :, 0:W - 1], ALU.subtract)
                    wxb = (wxr[:, i, 0:W - 1].unsqueeze(1)
                           .broadcast_to([H, C // 2, W - 1]))
                    nc.vector.tensor_tensor(tv[:, :, 0:W - 1],
                                            tv[:, :, 0:W - 1],
                                            wxb, ALU.mult)

            for g in range(NG):
                i, cg = divmod(g, C // GCH)
                off = g * GCOLS
                xg = xs[:, off:off + GCOLS]
                # psumA = Sy @ x
                ga = pa.tile([H, GCOLS], f32, tag="ga")
                nc.tensor.matmul(ga[:, 0:512], stat_sy, xg[:, 0:512],
                                 start=True, stop=True)
                nc.tensor.matmul(ga[:, 512:GCOLS], stat_sy, xg[:, 512:GCOLS],
                                 start=True, stop=True)
                # t2 = psumA * wy   (broadcast over GCH channels)
                # (GPSIMD cannot read PSUM on HW: evac via Act first)
                a0 = t2p.tile([H, GCOLS], bf16, tag="a0")
                nc.scalar.copy(a0[:], ga[:])
                t2 = t2p.tile([H, GCOLS], bf16, tag="t2")
                wyb = (wyr[:, i].unsqueeze(1).broadcast_to([H, GCH, W]))
                nc.gpsimd.tensor_tensor(
                    t2[:].rearrange('h (c w) -> h c w', c=GCH),
                    a0[:].rearrange('h (c w) -> h c w', c=GCH),
                    wyb, ALU.mult)
                # psumB = I@b [+ I@t_cur] - I@x - I@t_prev - SyT@t2
                # (items 2,3: t_cur joined on DVE instead, to balance PE)
                on_pe = (i < BL // 2)
                b_pe = True
                gb = pb.tile([H, GCOLS], f32, tag="gb")
                for lo, hi in ((0, 512), (512, GCOLS)):
                    first = True
                    if b_pe:
                        nc.tensor.matmul(gb[:, lo:hi], stat_i,
                                         bt[:, off + lo:off + hi],
                                         start=True, stop=False)
                        first = False
                    if on_pe:
                        nc.tensor.matmul(gb[:, lo:hi], stat_i,
                                         tp[:, 1 + off + lo:1 + off + hi],
                                         start=first, stop=False)
                        first = False
                    nc.tensor.matmul(gb[:, lo:hi], stat_ni,
                                     xs[:, off + lo:off + hi],
                                     start=first, stop=False)
                    nc.tensor.matmul(gb[:, lo:hi], stat_ni,
                                     tp[:, off + lo:off + hi],
                                     start=False, stop=False)
                    nc.tensor.matmul(gb[:, lo:hi], stat_nsyt, t2[:, lo:hi],
                                     start=False, stop=True)
                # evac (bf16); off-PE joins on DVE
                nc.scalar.copy(ut[:, off:off + GCOLS], gb[:])
                if not on_pe:
                    nc.vector.tensor_tensor(
                        ut[:, off:off + GCOLS], ut[:, off:off + GCOLS],
                        tp[:, 1 + off:1 + off + GCOLS], ALU.add)
                if not b_pe:
                    nc.vector.tensor_tensor(
                        ut[:, off:off + GCOLS], ut[:, off:off + GCOLS],
                        bt[:, off:off + GCOLS], ALU.add)

            # z = r * (om*gam*dinv) [Pool]; d = (om-1)*d [Pool]; d += z [DVE]
            HC = GCH
            omb = omt[:, k:k + 1].broadcast_to([H, GCOLS])
            for i in range(BL):
                for h2 in range(C // GCH):
                    sl = slice(i * ITEM + h2 * GCOLS,
                               i * ITEM + (h2 + 1) * GCOLS)
                    dkb = (dkr[:, k + 1, i].unsqueeze(1)
                           .broadcast_to([H, HC, W]))
                    nc.gpsimd.tensor_tensor(
                        mt[:, sl].rearrange('h (c w) -> h c w', c=HC),
                        ut[:, sl].rearrange('h (c w) -> h c w', c=HC),
                        dkb, ALU.mult)
                    nc.gpsimd.tensor_tensor(dt[:, sl], dt[:, sl], omb,
                                            ALU.mult)
                    nc.vector.tensor_tensor(dt[:, sl], dt[:, sl],
                                            mt[:, sl], ALU.add)

            # x_{k+1} = x_k + d   (in-place DMA accumulate; last iter -> f32)
            for i in range(BL):
                if last:
                    nc.vector.tensor_tensor(item_view(ot, i), item_view(xs, i),
                                            item_view(dt, i), ALU.add)
                    o4 = ot[:].rearrange('h (b c w) -> h b c w', b=BL, c=C)
                    nc.sync.dma_start(out_ap[:, i], o4[:, i])
                else:
                    for h2 in range(2):
                        sl = slice(i * ITEM + h2 * (ITEM // 2),
                                   i * ITEM + (h2 + 1) * (ITEM // 2))
                        nc.vector.tensor_tensor(xa[:, sl], xa[:, sl],
                                                dt[:, sl], ALU.add)

    nc.compile()
    return nc


def _get_program(K):
    # program is data-independent (all per-iteration scalars arrive via the
    # omv/dinvk/stats input tensors) -> compile once per K
    if K not in _cache:
        _cache[K] = _build(K)
    return _cache[K]


def _host_prep(ae, wxwy, K):
    """Spectral bounds, schedules, per-core h-major bf16 inputs."""
    import ml_dtypes
    bf = ml_dtypes.bfloat16
    ae = np.ascontiguousarray(ae, np.float32)
    wxwy = np.ascontiguousarray(wxwy, np.float32)
    wx = wxwy[:, 0]
    wy = wxwy[:, 1]

    d = np.ones((B, H, W), np.float32)
    d[:, :, 1:] += wx[:, :, :-1]
    d[:, :, :-1] += wx[:, :, :-1]
    d[:, 1:, :] += wy[:, :-1, :]
    d[:, :-1, :] += wy[:, :-1, :]
    dinv = 1.0 / d
    dis = np.sqrt(dinv)

    def op_precond(v):  # D^-1/2 A D^-1/2, v: [B,H,W]
        u = dis * v
        dx = u[:, :, 1:] - u[:, :, :-1]
        dy = u[:, 1:, :] - u[:, :-1, :]
        wdx = wx[:, :, :-1] * dx
        wdy = wy[:, :-1, :] * dy
        out = u.copy()
        out[:, :, 1:] += wdx
        out[:, :, :-1] -= wdx
        out[:, 1:, :] += wdy
        out[:, :-1, :] -= wdy
        return dis * out

    rng = np.random.default_rng(3)
    v = rng.standard_normal((B, H, W)).astype(np.float32)
    for _ in range(30):
        av = op_precond(v)
        v = av / np.sqrt((av * av).sum(axis=(1, 2), keepdims=True))
    lmax = float(((v * op_precond(v)).sum(axis=(1, 2))).max())
    s = lmax + 0.05
    v = rng.standard_normal((B, H, W)).astype(np.float32)
    for _ in range(40):
        av = s * v - op_precond(v)
        v = av / np.sqrt((av * av).sum(axis=(1, 2), keepdims=True))
    lmin = s - float(((v * (s * v - op_precond(v))).sum(axis=(1, 2))).max())
    lmax *= 1.005
    lmin = max(1.0 / float(d.max()), lmin * 0.995)

    gam, oms = _cheb_omegas(lmin, lmax, K)
    stats = _host_stats(oms, K)

    # dinvk slots: 0 -> gamma*dinv, k+1 -> om_k*gamma*dinv   [B,K+1,H,W]
    dk = np.empty((B, K + 1, H, W), np.float32)
    dk[:, 0] = gam * dinv
    for k in range(K):
        dk[:, k + 1] = oms[k] * gam * dinv

    # per-partition scalars (om_k - 1) for the d-update stt
    omv = np.tile(np.float32([om - 1.0 for om in oms]), (H, 1))

    # h-major transposes
    bh = np.ascontiguousarray(ae.transpose(2, 0, 1, 3)).astype(bf)  # [H,B,C,W]
    wxh = np.ascontiguousarray(wx.transpose(1, 0, 2)).astype(bf)    # [H,B,W]
    wyh = np.ascontiguousarray(wy.transpose(1, 0, 2)).astype(bf)
    dkh = np.ascontiguousarray(dk.transpose(2, 1, 0, 3)).astype(bf)  # [H,K+1,B,W]

    in_maps = []
    for c in range(NCORES):
        sl = slice(c * BL, (c + 1) * BL)
        in_maps.append({
            "bh": np.ascontiguousarray(bh[:, sl]),
            "wxh": np.ascontiguousarray(wxh[:, sl]),
            "wyh": np.ascontiguousarray(wyh[:, sl]),
            "dinvk": np.ascontiguousarray(dkh[:, :, sl]),
            "stats": stats,
            "omv": omv,
        })
    return in_maps, oms


def postprocess_core(out_core):
    """[H,BL,C,W] f32 -> [BL,C,H,W]"""
    return np.ascontiguousarray(out_core.transpose(1, 2, 0, 3))


def prepare(ae, wxwy):
    in_maps, oms = _host_prep(ae, wxwy, K_ITERS)
    nc = _get_program(K_ITERS)
    return {"nc": nc, "in_maps": in_maps,
            "postprocess_core": lambda o, c: postprocess_core(o)}


def kernel(ae, wxwy):
    import sys
    if '/opt/trn_rl_repo' not in sys.path:
        sys.path.insert(0, '/opt/trn_rl_repo')
    from concourse.bass_utils import run_bass_kernel_spmd

    p = prepare(ae, wxwy)
    res = run_bass_kernel_spmd(p["nc"], p["in_maps"], list(range(NCORES)))
    out = np.concatenate(
        [postprocess_core(np.asarray(res.results[c]["out"]).reshape(H, BL, C, W))
         for c in range(NCORES)], axis=0)
    return out.astype(np.float32)
